# revision 23
# baseline (speedup 1.0000x reference)
"""GAT layer (nn_GATLayer_44220983279640) — Trainium2 Bass/Tile kernel.

Reference math per graph (B=16, D=512, FIN=FOUT=128, H=8):
    h  = x @ W                                         [D, F]
    s1[hd,i] = h[i] . a1[hd]   s2[hd,j] = h[j] . a2[hd]
    e  = leaky_relu(s1[:,None] + s2[None,:] + ab)      [H, D, D]
    att = softmax_j(where(adj > 0, e, -9e15))
    out = mean_hd(att @ h)                             [D, F]

Sharding: data-parallel over batch, 2 graphs per core on 8 cores.

Device strategy v2 (all-fp16 elementwise path):
  * E^T[j, i] layout; additive fp16 mask (adj>0 ? 0 : -6e4) prepared host-side.
  * v = (maskT + s2[j]) + s1b on DVE as 4 chunk STTs, all operands fp16 so
    the DVE runs 2x_1P mode (halves the f32 baseline cost).
  * leaky_relu on DVE as ONE wide STT: u = max(v*0.01, v) — frees an entire
    ACT pass per head-graph vs Prelu on ACT.
  * exp on ACT, one wide [128, 2048] fp16 pass, shifted per head so outputs
    are in (0, e^8] (fp16-normal); shift errors cancel in softmax exactly.
  * agg matmuls: psum[i-tile] += E^T-slice^T @ [h/8 | ones] (fp16 weights).
  * normalize-evict on ACT: Copy(psum * rcol) with per-partition scale AP;
    head-accumulate as ONE wide fp16 tensor_tensor add on DVE.
  * fp16 setup matmuls (1 cyc/row vs 4 for f32), one batched mask DMA per
    graph, fp16 output (host upcasts to f32).
"""

from contextlib import ExitStack

import numpy as np

import concourse.bass as bass
import concourse.bacc as bacc
import concourse.tile as tile
from concourse import mybir
from concourse import dve_ops as _dvo
from concourse.bass_utils import run_bass_kernel_spmd
from concourse.dve_spec import C0, C2, Spec, Src0, Src1, lower, maxx
from concourse.dve_uop import DveOpSpec


def _register_vlrelu():
    """Custom fused DVE op: out = leaky_relu(in0 + s0 + in1).

    One 1x DVE pass replaces the STT (mask + s2 + s1b) AND the leaky_relu
    pass (scalar_tensor_tensor has no 2x uop, so two stock passes would cost
    ~2x this single fused op)."""
    name = "GAT_VLRELU_ANT"
    for op in _dvo.OPS:
        if op.name == name:
            return op
    x = (Src0 + C0) + Src1

    def _ref(in0, in1, c0, c1, c2):
        y = in0.astype(np.float32) + c0 + in1.astype(np.float32)
        return np.maximum(y, y * c2)

    spec = Spec(body=maxx(x, x * C2), reference=_ref)
    row = _dvo._CUSTOM_DVE_ROW_BASE + len(_dvo.OPS)
    shas = {}
    for ver in ("v3", "v4"):
        try:
            uops = lower(spec, ver=ver)
            shas[ver] = DveOpSpec(
                name=name, opcode=row, uops=uops, rd1_en=True
            ).sha(ver)
        except Exception:
            pass
    op = _dvo.DveOp(name, spec, subdim=False, uops_sha=shas)
    _dvo.OPS.append(op)
    _dvo._SUB_OPCODE_FOR_NAME[name] = row
    return op


VLRELU = _register_vlrelu()


def _vlrelu_uops_2x():
    """Hand-built 2x_1P uop program for VLRELU (lower() only emits 1x).

    Packed fp16 pairs: blocks 0-3 compute the lo element, blocks 4-7 the hi
    element. Inputs ride delay lanes (input lane k+1 <-> delay lane k):
    D0=SRC_0 D1=CONST_0 D2=SRC_1 D3=CONST_2 D4=SRC_0_HI D5=SRC_1_HI.
    u_lo is captured into D0 at block 4 and written from DELAY_0; u_hi is
    block 7's ALU_OUT."""
    from concourse.dve_uop import (
        AluInp, AluOp, DelayInp, InpSel, OutPath, OutSel, Trigger,
        UopConfig, UopDpConfig,
    )

    def blk(op, a, b, cap=None):
        d = UopDpConfig(
            op=op, alu_src0=a, alu_src1=b,
            delay=[DelayInp.PREV_DELAY] * 7,
            alu_out_enable=1,
            delay_enable=[1, 1, 1, 1, 1, 1, 0],
        )
        if cap is not None:
            d.delay[cap] = DelayInp.PREV_ALU_OUT
        return d

    A = AluInp
    dp = [
        blk(AluOp.ADD, A.PREV_DELAY_0, A.PREV_DELAY_1),           # S0+C0
        blk(AluOp.ADD, A.PREV_ALU_OUT, A.PREV_DELAY_2),           # +S1 = x_lo
        blk(AluOp.MULTIPLY, A.PREV_ALU_OUT, A.PREV_DELAY_3, cap=0),  # x_lo*C2; D0<-x_lo
        blk(AluOp.MAX, A.PREV_DELAY_0, A.PREV_ALU_OUT),           # u_lo
        blk(AluOp.ADD, A.PREV_DELAY_4, A.PREV_DELAY_1, cap=0),    # S0H+C0; D0<-u_lo
        blk(AluOp.ADD, A.PREV_ALU_OUT, A.PREV_DELAY_5),           # +S1H = x_hi
        blk(AluOp.MULTIPLY, A.PREV_ALU_OUT, A.PREV_DELAY_3, cap=1),  # x_hi*C2; D1<-x_hi
        blk(AluOp.MAX, A.PREV_DELAY_1, A.PREV_ALU_OUT),           # u_hi
    ]
    u = UopConfig(
        inp=[InpSel.ZERO, InpSel.SRC_0, InpSel.CONST_0, InpSel.SRC_1,
             InpSel.CONST_2, InpSel.SRC_0_HI, InpSel.SRC_1_HI, InpSel.ZERO],
        inp_enable=[0, 1, 1, 1, 1, 1, 1, 0],
        out={OutPath.WR0_LO: OutSel.DELAY_0, OutPath.WR0_HI: OutSel.ALU_OUT,
             OutPath.WR1_LO: OutSel.ALU_OUT, OutPath.WR1_HI: OutSel.ALU_OUT},
        out_enable={OutPath.WR0_LO: 1, OutPath.WR0_HI: 1,
                    OutPath.WR1_LO: 0, OutPath.WR1_HI: 0},
        require_inp0=1, require_inp1=1,
        trigger=(Trigger.SRC_TENSOR_DONE, Trigger.NONE, Trigger.NONE),
        datapath_config=dp,
    )
    return [u]


def _inject_vlrelu_2x():
    """Swap the compiled spec for VLRELU with one carrying the 2x program.

    DveOp.compile() is memoised in dve_ops._COMPILE_CACHE; seeding the cache
    makes dve_table_for_ops pick up the augmented table."""
    from concourse.dve_table_gen import dve_ver_for

    ver = dve_ver_for("TRN2")
    spec1x = lower(VLRELU.spec, ver=ver)
    augmented = DveOpSpec(
        name=VLRELU.name,
        opcode=_dvo.get_dve_sub_opcode(VLRELU.name),
        uops=spec1x,
        uops_2x=_vlrelu_uops_2x(),
        rd1_en=True,
        perf_max=1,
    )
    augmented.validate(ver)
    _dvo._COMPILE_CACHE[(VLRELU.name, ver)] = augmented


_inject_vlrelu_2x()


def _emit_vlrelu(nc, out, in0, in1, s0, imm2):
    """nc.vector._custom_dve for VLRELU, but with perf_max=1 so the engine
    may select the 2x_1P table slot when operands are packed fp16."""
    import concourse.bass_isa as bass_isa

    v = nc.vector
    if VLRELU.name not in v.bass.m.ant_custom_dve_ops:
        v.bass.m.ant_custom_dve_ops = sorted(
            {*v.bass.m.ant_custom_dve_ops, VLRELU.name}
        )
    shape = bass_isa.CustomDveShape.TTSS
    isa_opcode = v.bass.isa.Opcode[
        f"NEURON_ISA_TPB_OPCODE_CUSTOM_DVE_ANT_{shape.slot()}"
    ].value
    ins = [
        v.lower_ap(in0, for_isa=True),
        v.lower_ap(in1, for_isa=True),
        v.lower_ap(s0, for_isa=True),
        mybir.ImmediateValue(dtype=mybir.dt.float32, value=0.0),
    ]
    outs = [v.lower_ap(out, for_isa=True)]
    return v.add_instruction(
        bass_isa.InstCustomDveAnt(
            name=v.bass.get_next_instruction_name(),
            op_name=VLRELU.name,
            rd1_en=True,
            subdim=0,
            imm2=imm2,
            shape=shape,
            row=_dvo.get_dve_sub_opcode(VLRELU.name),
            isa_opcode=isa_opcode,
            perf_max=1,
            ins=ins,
            outs=outs,
        )
    )

B, D, FIN, FOUT, H = 16, 512, 128, 128, 8
NCORES = 8
NB = B // NCORES          # graphs per core
P = 128                   # partitions
NCH = D // P              # 4 j-chunks / i-tiles
NEGM = -60000.0           # fp16-safe "masked" logit

F32 = mybir.dt.float32
F16 = mybir.dt.float16

# packed fp16 consts layout (columns): W | W^T | aT | ab | id8 | ones | zero
C_W = 0
C_WT = FOUT
C_AT = 2 * FOUT
C_AB = 2 * FOUT + 2 * H
C_ID8 = C_AB + 1
C_ONES = C_ID8 + H
C_I128 = C_ONES + P
C_ZERO = C_I128 + P
CONST_COLS = C_ZERO + 1

_NC_CACHE = {}


def _build_bass():
    nc = bacc.Bacc("TRN2", debug=False, num_devices=NCORES)

    xT = nc.dram_tensor("xT", [NB, FIN, D], F16, kind="ExternalInput").ap()
    maskT = nc.dram_tensor("maskT", [NB, P, NCH * D], F16, kind="ExternalInput").ap()
    consts = nc.dram_tensor("consts", [P, CONST_COLS], F16, kind="ExternalInput").ap()
    s1d = nc.dram_tensor("s1d", [NB, H, D], F16).ap()
    out = nc.dram_tensor("out", [NB, D, FOUT], F16, kind="ExternalOutput").ap()

    with tile.TileContext(nc) as tc, ExitStack() as ctx:
        _kernel_body(ctx, tc, out, xT, maskT, consts, s1d)
    nc.compile()
    return nc


def _kernel_body(ctx, tc, out, xT, maskT, consts, s1d):
    nc = tc.nc
    add, mult, amax = mybir.AluOpType.add, mybir.AluOpType.mult, mybir.AluOpType.max

    const = ctx.enter_context(tc.tile_pool(name="const", bufs=1))
    xpool = ctx.enter_context(tc.tile_pool(name="xpool", bufs=NB))
    mpool = ctx.enter_context(tc.tile_pool(name="mpool", bufs=NB))
    spool = ctx.enter_context(tc.tile_pool(name="spool", bufs=NB))
    s2tpool = ctx.enter_context(tc.tile_pool(name="s2tpool", bufs=2 * NCH))
    upool = ctx.enter_context(tc.tile_pool(name="upool", bufs=5))
    epool = ctx.enter_context(tc.tile_pool(name="epool", bufs=5))
    s1bpool = ctx.enter_context(tc.tile_pool(name="s1bpool", bufs=6))
    hpool = ctx.enter_context(tc.tile_pool(name="hpool", bufs=2 * NCH))
    apool = ctx.enter_context(tc.tile_pool(name="apool", bufs=2))
    evpool = ctx.enter_context(tc.tile_pool(name="evpool", bufs=6))
    npool = ctx.enter_context(tc.tile_pool(name="npool", bufs=4))
    # PSUM: 2 (setup scratch) + 6 (agg out; 2 i-tiles per bank) = 8 banks
    pset = ctx.enter_context(tc.tile_pool(name="pset", bufs=2, space="PSUM"))
    pout = ctx.enter_context(tc.tile_pool(name="pout", bufs=4, space="PSUM"))

    # --- constants (one packed DMA) ----------------------------------------
    cst = const.tile([P, CONST_COLS], F16)
    nc.sync.dma_start(out=cst, in_=consts)
    W_sb = cst[:, C_W : C_W + FOUT]
    WT_sb = cst[:, C_WT : C_WT + FOUT]
    aT_sb = cst[:, C_AT : C_AT + 2 * H]
    ab_sb = cst[0:H, C_AB : C_AB + 1]
    ident8 = cst[0:H, C_ID8 : C_ID8 + H]
    onesrow = cst[0:1, C_ONES : C_ONES + P]
    ident128 = cst[:, C_I128 : C_I128 + P]

    # GPSIMD ucode warmup: first use of a freshly-loaded Q7 kernel pays a
    # ~6us IRAM load; trigger it during the prologue on dummy tiles.
    wrm = const.tile([P, 2], F32)
    nc.vector.memset(wrm[:], 1.0)
    nc.gpsimd.normalize_recip(wrm[:, 0:1], wrm[:, 0:1], wrm[:, 1:2])
    nc.gpsimd.tensor_add(wrm[:, 0:1], wrm[:, 0:1], wrm[:, 1:2])

    # Wa[fin, 0:8]=W@a1^T, [fin, 8:16]=W@a2^T  (shared across graphs)
    p_wa = pset.tile([P, D], F32, tag="setup")
    nc.tensor.matmul(p_wa[:, 0 : 2 * H], WT_sb, aT_sb, start=True, stop=True)
    Wa_sb = const.tile([FIN, 2 * H], F16)
    nc.vector.tensor_copy(Wa_sb[:], p_wa[:, 0 : 2 * H])
    ab32 = const.tile([H, 1], F32)
    nc.vector.tensor_copy(ab32[:], ab_sb)

    G = []  # per-graph setup state
    for b in range(NB):
        # --- per-graph setup ----------------------------------------------
        x_sb = xpool.tile([FIN, D], F16, tag="x")
        nc.sync.dma_start(out=x_sb, in_=xT[b])
        m_sb = mpool.tile([P, NCH * D], F16, tag="mask")
        qs = [nc.scalar, nc.gpsimd, nc.sync]
        for c in range(NCH):
            qs[(b * NCH + c) % 3].dma_start(
                out=m_sb[:, bass.ts(c, D)], in_=maskT[b][:, bass.ts(c, D)]
            )

        # s1/s2 for all heads: [8, D] each
        p_s1 = pset.tile([P, D], F32, tag="setup")
        nc.tensor.matmul(p_s1[0:H, :], Wa_sb[:, 0:H], x_sb[:], start=True, stop=True)
        s1_sb = spool.tile([H, D], F16, tag="s1")
        nc.vector.tensor_copy(s1_sb[:], p_s1[0:H, :])
        # stage s1 rows in DRAM; the head loop row-broadcasts them back via DMA
        nc.scalar.dma_start(out=s1d[b], in_=s1_sb[:])
        mx1 = spool.tile([H, 1], F32, tag="mx1")
        nc.vector.reduce_max(
            out=mx1[:], in_=p_s1[0:H, :], axis=mybir.AxisListType.X, negate=True
        )

        p_s2 = pset.tile([P, D], F32, tag="setup")
        nc.tensor.matmul(
            p_s2[0:H, :], Wa_sb[:, H : 2 * H], x_sb[:], start=True, stop=True
        )
        s2b_sb = spool.tile([H, D], F16, tag="s2")
        nc.vector.tensor_scalar(
            out=s2b_sb[:], in0=p_s2[0:H, :], scalar1=ab32[:], scalar2=None,
            op0=add,
        )
        mx2 = spool.tile([H, 1], F32, tag="mx2")
        nc.vector.reduce_max(
            out=mx2[:], in_=s2b_sb[:], axis=mybir.AxisListType.X, negate=True
        )

        # Per-head negated logit upper bound + 8: exp bias (softmax-shift
        # errors cancel per head, so fp16 is fine here).
        nbound = spool.tile([H, 1], F32, tag="nbound")
        nc.vector.tensor_add(nbound[:], mx1[:], mx2[:])
        nc.vector.tensor_scalar_add(nbound[:], nbound[:], 8.0)
        nb16 = spool.tile([H, 1], F16, tag="nb16")
        nc.vector.tensor_copy(nb16[:], nbound[:])
        # broadcast -bound to [P, H] columns: transpose to a row, then
        # ones-row outer-product
        p_nt = pset.tile([P, D], F32, tag="setup")
        nc.tensor.matmul(p_nt[0:1, 0:H], nb16[:], ident8, start=True, stop=True)
        nbT = spool.tile([1, H], F16, tag="nbT")
        nc.vector.tensor_copy(nbT[:], p_nt[0:1, 0:H])
        p_nb = pset.tile([P, D], F32, tag="setup")
        nc.tensor.matmul(p_nb[:, 0:H], onesrow, nbT[:], start=True, stop=True)
        nbcols = spool.tile([P, H], F16, tag="nbcols")
        nc.vector.tensor_copy(nbcols[:], p_nb[:, 0:H])

        # s2b columns: [P, H] per j-chunk (PE transpose of [8, 128] slices)
        s2bT = []
        for c in range(NCH):
            p_t = pset.tile([P, D], F16, tag="setup")
            nc.tensor.transpose(p_t[:, 0:H], s2b_sb[:, bass.ts(c, P)], ident8)
            st = s2tpool.tile([P, H], F32, tag="s2T")
            nc.vector.tensor_copy(st[:], p_t[:, 0:H])
            s2bT.append(st)

        # h tiles + ones column, fp16, h pre-scaled by 1/H
        haug = []
        for c in range(NCH):
            p_h = pset.tile([P, D], F32, tag="setup")
            nc.tensor.matmul(
                p_h[:, 0:FOUT], x_sb[:, bass.ts(c, P)], W_sb, start=True, stop=True
            )
            ha = hpool.tile([P, FOUT + 1], F16, tag="haug")
            nc.scalar.activation(
                ha[:, 0:FOUT], p_h[:, 0:FOUT],
                mybir.ActivationFunctionType.Copy, scale=1.0 / H,
            )
            nc.vector.memset(ha[:, FOUT : FOUT + 1], 1.0)
            haug.append(ha)

        # head-0 s1b via PE row-broadcast (skips the DRAM round-trip latency
        # that would otherwise gate the first head-graph)
        p_b = pset.tile([P, D], F32, tag="setup")
        nc.tensor.matmul(p_b[:], onesrow, s1_sb[0:1, :], start=True, stop=True)
        s1b0 = s1bpool.tile([P, D], F16, tag="s1b")
        nc.scalar.activation(s1b0[:], p_b[:], mybir.ActivationFunctionType.Copy)
        G.append(dict(m_sb=m_sb, s2bT=s2bT, haug=haug, nbcols=nbcols,
                      s1b0=s1b0))

    # per-graph head-accumulators: one full PSUM bank each, reusing the two
    # setup banks (setup is complete by the first write)
    for b in range(NB):
        acc_ps = pset.tile([P, NCH * FOUT], F32, tag="accps")
        G[b]["acc_ps"] = acc_ps

    # --- main per-head loop, graphs interleaved for deeper ILP ------------
    for hd in range(H):
        for b in range(NB):
            m_sb, s2bT = G[b]["m_sb"], G[b]["s2bT"]
            haug, nbcols = G[b]["haug"], G[b]["nbcols"]
            if hd == 0:
                s1b = G[b]["s1b0"]
            else:
                # S1B = s1 row hd broadcast across partitions (DMA row-bcast)
                s1b = s1bpool.tile([P, D], F16, tag="s1b")
                s1row = s1d[b, hd]
                nc.sync.dma_start(
                    out=s1b[:],
                    in_=bass.AP(
                        tensor=s1d.tensor, offset=s1row.offset,
                        ap=[[0, P], s1row.ap[-1]],
                    ),
                )

            # u = leaky_relu(maskT + s2b[j] + S1B): one fused custom DVE op
            # per j-chunk (replaces STT + separate lrelu pass)
            u = upool.tile([P, NCH * D], F16, tag="u")
            for c in range(NCH):
                _emit_vlrelu(
                    nc,
                    out=u[:, bass.ts(c, D)],
                    in0=m_sb[:, bass.ts(c, D)],
                    in1=s1b[:],
                    s0=s2bT[c][:, hd : hd + 1],
                    imm2=0.01,
                )
            # E = exp(u - bound + 8), one wide fp16 ACT pass
            E = epool.tile([P, NCH * D], F16, tag="E")
            nc.scalar.activation(
                E[:], u[:], mybir.ActivationFunctionType.Exp,
                bias=nbcols[:, hd : hd + 1],
            )

            # agg: psum[i-tile t] += E^T[:, t]^T @ [h/8 | 1]; two i-tiles
            # share a PSUM bank so the f32 evict is 2 wide ACT copies.
            TW = FOUT + 1
            p_bank = []
            for t2 in range(NCH // 2):
                p_o = pout.tile([P, 2 * TW], F32, tag="po")
                for half in range(2):
                    t = 2 * t2 + half
                    for c in range(NCH):
                        nc.tensor.matmul(
                            p_o[:, half * TW : (half + 1) * TW],
                            E[:, c * D + t * P : c * D + (t + 1) * P],
                            haug[c][:],
                            start=(c == 0),
                            stop=(c == NCH - 1),
                        )
                p_bank.append(p_o)
            # batched strided recip per PSUM bank (rowsum cols 128, 257);
            # normalize-evict split DVE/ACT; head-accumulate on GPSIMD
            rcs = []
            for t2 in range(NCH // 2):
                rc = npool.tile([P, 2], F32, tag="rc")
                p_o = p_bank[t2]
                nc.vector.reciprocal(
                    rc[:],
                    bass.AP(
                        tensor=p_o.tensor, offset=p_o[:, FOUT : FOUT + 1].offset,
                        ap=[p_o.ap[0], [TW, 2]],
                    ),
                )
                rcs.append(rc)
            tgt = evpool.tile([P, NCH * FOUT], F16, tag="ev")
            for t in range(NCH):
                p_o = p_bank[t // 2]
                half = t % 2
                if t < 3:
                    nc.vector.tensor_scalar(
                        out=tgt[:, bass.ts(t, FOUT)],
                        in0=p_o[:, half * TW : half * TW + FOUT],
                        scalar1=rcs[t // 2][:, half : half + 1],
                        scalar2=None, op0=mult,
                    )
                else:
                    nc.scalar.activation(
                        tgt[:, bass.ts(t, FOUT)],
                        p_o[:, half * TW : half * TW + FOUT],
                        mybir.ActivationFunctionType.Copy,
                        scale=rcs[t // 2][:, half : half + 1],
                    )
            # head-accumulate in PSUM: acc += I @ ev (one matmul, off the
            # saturated DVE/ACT engines)
            nc.tensor.matmul(
                G[b]["acc_ps"][:], ident128, tgt[:],
                start=(hd == 0), stop=(hd == H - 1),
            )

    for b in range(NB):
        # out[b, t*128+p, f] = acc[p, t*FOUT+f]
        acc_sb = apool.tile([P, NCH * FOUT], F16, tag="acc")
        nc.scalar.activation(
            acc_sb[:], G[b]["acc_ps"][:], mybir.ActivationFunctionType.Copy
        )
        nc.sync.dma_start(
            out=bass.AP(
                tensor=out.tensor, offset=out[b].offset,
                ap=[[FOUT, P], [P * FOUT, NCH], [1, FOUT]],
            ),
            in_=acc_sb[:],
        )


def _prep_core_inputs(input, adj, W, a_w, a_b, core):
    gs = slice(core * NB, (core + 1) * NB)
    x_c = input[gs]                                   # [NB, D, FIN]
    adj_c = adj[gs]                                   # [NB, D, D] int32
    xT = np.ascontiguousarray(x_c.transpose(0, 2, 1)).astype(np.float16)
    adjT = adj_c.transpose(0, 2, 1)                   # [NB, j, i]

    maskT = np.where(adjT > 0, np.float16(0.0), np.float16(NEGM))
    # [NB, j, i] -> [NB, NCH, P, i] -> [NB, P, NCH, i] -> [NB, P, NCH*D]
    maskT = np.ascontiguousarray(
        maskT.reshape(NB, NCH, P, D).transpose(0, 2, 1, 3).reshape(NB, P, NCH * D)
    )
    return {
        "xT": xT,
        "maskT": maskT,
        "consts": _pack_consts(W, a_w, a_b),
    }


def _pack_consts(W, a_w, a_b):
    c = np.zeros((P, CONST_COLS), dtype=np.float16)
    c[:, C_W : C_W + FOUT] = W
    c[:, C_WT : C_WT + FOUT] = W.T
    c[:, C_AT : C_AT + H] = a_w[:, :FOUT].T
    c[:, C_AT + H : C_AT + 2 * H] = a_w[:, FOUT:].T
    c[0:H, C_AB] = a_b
    c[0:H, C_ID8 : C_ID8 + H] = np.eye(H)
    c[0:1, C_ONES : C_ONES + P] = 1.0
    c[:, C_I128 : C_I128 + P] = np.eye(P)
    return c


def get_nc():
    if "nc" not in _NC_CACHE:
        _NC_CACHE["nc"] = _build_bass()
    return _NC_CACHE["nc"]


def run_on_device(in_maps, **kwargs):
    return run_bass_kernel_spmd(get_nc(), in_maps, list(range(NCORES)), **kwargs)


def kernel(input, adj, W, a_w, a_b):
    input = np.asarray(input, dtype=np.float32)
    adj = np.asarray(adj)
    W = np.asarray(W, dtype=np.float32)
    a_w = np.asarray(a_w, dtype=np.float32)
    a_b = np.asarray(a_b, dtype=np.float32)

    in_maps = [
        _prep_core_inputs(input, adj, W, a_w, a_b, c) for c in range(NCORES)
    ]
    res = run_on_device(in_maps)
    outs = [res.results[c]["out"] for c in range(NCORES)]
    return np.concatenate(outs, axis=0).astype(np.float32)


if __name__ == "__main__":
    nc = get_nc()
    print("built ok")


# revision 24
# speedup vs baseline: 1.0102x; 1.0102x over previous
"""GAT layer (nn_GATLayer_44220983279640) — Trainium2 Bass/Tile kernel.

Reference math per graph (B=16, D=512, FIN=FOUT=128, H=8):
    h  = x @ W                                         [D, F]
    s1[hd,i] = h[i] . a1[hd]   s2[hd,j] = h[j] . a2[hd]
    e  = leaky_relu(s1[:,None] + s2[None,:] + ab)      [H, D, D]
    att = softmax_j(where(adj > 0, e, -9e15))
    out = mean_hd(att @ h)                             [D, F]

Sharding: data-parallel over batch, 2 graphs per core on 8 cores.

Device strategy v2 (all-fp16 elementwise path):
  * E^T[j, i] layout; additive fp16 mask (adj>0 ? 0 : -6e4) prepared host-side.
  * v = (maskT + s2[j]) + s1b on DVE as 4 chunk STTs, all operands fp16 so
    the DVE runs 2x_1P mode (halves the f32 baseline cost).
  * leaky_relu on DVE as ONE wide STT: u = max(v*0.01, v) — frees an entire
    ACT pass per head-graph vs Prelu on ACT.
  * exp on ACT, one wide [128, 2048] fp16 pass, shifted per head so outputs
    are in (0, e^8] (fp16-normal); shift errors cancel in softmax exactly.
  * agg matmuls: psum[i-tile] += E^T-slice^T @ [h/8 | ones] (fp16 weights).
  * normalize-evict on ACT: Copy(psum * rcol) with per-partition scale AP;
    head-accumulate as ONE wide fp16 tensor_tensor add on DVE.
  * fp16 setup matmuls (1 cyc/row vs 4 for f32), one batched mask DMA per
    graph, fp16 output (host upcasts to f32).
"""

from contextlib import ExitStack

import numpy as np

import concourse.bass as bass
import concourse.bacc as bacc
import concourse.tile as tile
from concourse import mybir
from concourse import dve_ops as _dvo
from concourse.bass_utils import run_bass_kernel_spmd
from concourse.dve_spec import C0, C2, Spec, Src0, Src1, lower, maxx
from concourse.dve_uop import DveOpSpec


def _register_vlrelu():
    """Custom fused DVE op: out = leaky_relu(in0 + s0 + in1).

    One 1x DVE pass replaces the STT (mask + s2 + s1b) AND the leaky_relu
    pass (scalar_tensor_tensor has no 2x uop, so two stock passes would cost
    ~2x this single fused op)."""
    name = "GAT_VLRELU_ANT"
    for op in _dvo.OPS:
        if op.name == name:
            return op
    x = (Src0 + C0) + Src1

    def _ref(in0, in1, c0, c1, c2):
        y = in0.astype(np.float32) + c0 + in1.astype(np.float32)
        return np.maximum(y, y * c2)

    spec = Spec(body=maxx(x, x * C2), reference=_ref)
    row = _dvo._CUSTOM_DVE_ROW_BASE + len(_dvo.OPS)
    shas = {}
    for ver in ("v3", "v4"):
        try:
            uops = lower(spec, ver=ver)
            shas[ver] = DveOpSpec(
                name=name, opcode=row, uops=uops, rd1_en=True
            ).sha(ver)
        except Exception:
            pass
    op = _dvo.DveOp(name, spec, subdim=False, uops_sha=shas)
    _dvo.OPS.append(op)
    _dvo._SUB_OPCODE_FOR_NAME[name] = row
    return op


VLRELU = _register_vlrelu()


def _vlrelu_uops_2x():
    """Hand-built 2x_1P uop program for VLRELU (lower() only emits 1x).

    Packed fp16 pairs: blocks 0-3 compute the lo element, blocks 4-7 the hi
    element. Inputs ride delay lanes (input lane k+1 <-> delay lane k):
    D0=SRC_0 D1=CONST_0 D2=SRC_1 D3=CONST_2 D4=SRC_0_HI D5=SRC_1_HI.
    u_lo is captured into D0 at block 4 and written from DELAY_0; u_hi is
    block 7's ALU_OUT."""
    from concourse.dve_uop import (
        AluInp, AluOp, DelayInp, InpSel, OutPath, OutSel, Trigger,
        UopConfig, UopDpConfig,
    )

    def blk(op, a, b, cap=None):
        d = UopDpConfig(
            op=op, alu_src0=a, alu_src1=b,
            delay=[DelayInp.PREV_DELAY] * 7,
            alu_out_enable=1,
            delay_enable=[1, 1, 1, 1, 1, 1, 0],
        )
        if cap is not None:
            d.delay[cap] = DelayInp.PREV_ALU_OUT
        return d

    A = AluInp
    dp = [
        blk(AluOp.ADD, A.PREV_DELAY_0, A.PREV_DELAY_1),           # S0+C0
        blk(AluOp.ADD, A.PREV_ALU_OUT, A.PREV_DELAY_2),           # +S1 = x_lo
        blk(AluOp.MULTIPLY, A.PREV_ALU_OUT, A.PREV_DELAY_3, cap=0),  # x_lo*C2; D0<-x_lo
        blk(AluOp.MAX, A.PREV_DELAY_0, A.PREV_ALU_OUT),           # u_lo
        blk(AluOp.ADD, A.PREV_DELAY_4, A.PREV_DELAY_1, cap=0),    # S0H+C0; D0<-u_lo
        blk(AluOp.ADD, A.PREV_ALU_OUT, A.PREV_DELAY_5),           # +S1H = x_hi
        blk(AluOp.MULTIPLY, A.PREV_ALU_OUT, A.PREV_DELAY_3, cap=1),  # x_hi*C2; D1<-x_hi
        blk(AluOp.MAX, A.PREV_DELAY_1, A.PREV_ALU_OUT),           # u_hi
    ]
    u = UopConfig(
        inp=[InpSel.ZERO, InpSel.SRC_0, InpSel.CONST_0, InpSel.SRC_1,
             InpSel.CONST_2, InpSel.SRC_0_HI, InpSel.SRC_1_HI, InpSel.ZERO],
        inp_enable=[0, 1, 1, 1, 1, 1, 1, 0],
        out={OutPath.WR0_LO: OutSel.DELAY_0, OutPath.WR0_HI: OutSel.ALU_OUT,
             OutPath.WR1_LO: OutSel.ALU_OUT, OutPath.WR1_HI: OutSel.ALU_OUT},
        out_enable={OutPath.WR0_LO: 1, OutPath.WR0_HI: 1,
                    OutPath.WR1_LO: 0, OutPath.WR1_HI: 0},
        require_inp0=1, require_inp1=1,
        trigger=(Trigger.SRC_TENSOR_DONE, Trigger.NONE, Trigger.NONE),
        datapath_config=dp,
    )
    return [u]


def _inject_vlrelu_2x():
    """Swap the compiled spec for VLRELU with one carrying the 2x program.

    DveOp.compile() is memoised in dve_ops._COMPILE_CACHE; seeding the cache
    makes dve_table_for_ops pick up the augmented table."""
    from concourse.dve_table_gen import dve_ver_for

    ver = dve_ver_for("TRN2")
    spec1x = lower(VLRELU.spec, ver=ver)
    augmented = DveOpSpec(
        name=VLRELU.name,
        opcode=_dvo.get_dve_sub_opcode(VLRELU.name),
        uops=spec1x,
        uops_2x=_vlrelu_uops_2x(),
        rd1_en=True,
        perf_max=1,
    )
    augmented.validate(ver)
    _dvo._COMPILE_CACHE[(VLRELU.name, ver)] = augmented


_inject_vlrelu_2x()


def _emit_vlrelu(nc, out, in0, in1, s0, imm2):
    """nc.vector._custom_dve for VLRELU, but with perf_max=1 so the engine
    may select the 2x_1P table slot when operands are packed fp16."""
    import concourse.bass_isa as bass_isa

    v = nc.vector
    if VLRELU.name not in v.bass.m.ant_custom_dve_ops:
        v.bass.m.ant_custom_dve_ops = sorted(
            {*v.bass.m.ant_custom_dve_ops, VLRELU.name}
        )
    shape = bass_isa.CustomDveShape.TTSS
    isa_opcode = v.bass.isa.Opcode[
        f"NEURON_ISA_TPB_OPCODE_CUSTOM_DVE_ANT_{shape.slot()}"
    ].value
    ins = [
        v.lower_ap(in0, for_isa=True),
        v.lower_ap(in1, for_isa=True),
        v.lower_ap(s0, for_isa=True),
        mybir.ImmediateValue(dtype=mybir.dt.float32, value=0.0),
    ]
    outs = [v.lower_ap(out, for_isa=True)]
    return v.add_instruction(
        bass_isa.InstCustomDveAnt(
            name=v.bass.get_next_instruction_name(),
            op_name=VLRELU.name,
            rd1_en=True,
            subdim=0,
            imm2=imm2,
            shape=shape,
            row=_dvo.get_dve_sub_opcode(VLRELU.name),
            isa_opcode=isa_opcode,
            perf_max=1,
            ins=ins,
            outs=outs,
        )
    )

B, D, FIN, FOUT, H = 16, 512, 128, 128, 8
NCORES = 8
NB = B // NCORES          # graphs per core
P = 128                   # partitions
NCH = D // P              # 4 j-chunks / i-tiles
NEGM = -60000.0           # fp16-safe "masked" logit

F32 = mybir.dt.float32
F16 = mybir.dt.float16

# packed fp16 consts layout (columns): W | W^T | aT | ab | id8 | ones | zero
C_W = 0
C_WT = FOUT
C_AT = 2 * FOUT
C_AB = 2 * FOUT + 2 * H
C_ID8 = C_AB + 1
C_ONES = C_ID8 + H
C_I128 = C_ONES + P
C_ZERO = C_I128 + P
CONST_COLS = C_ZERO + 1

_NC_CACHE = {}


def _build_bass():
    nc = bacc.Bacc("TRN2", debug=False, num_devices=NCORES)

    xT = nc.dram_tensor("xT", [NB, FIN, D], F16, kind="ExternalInput").ap()
    maskT = nc.dram_tensor("maskT", [NB, P, NCH * D], F16, kind="ExternalInput").ap()
    consts = nc.dram_tensor("consts", [P, CONST_COLS], F16, kind="ExternalInput").ap()
    s1d = nc.dram_tensor("s1d", [NB, H, D], F16).ap()
    out = nc.dram_tensor("out", [NB, D, FOUT], F16, kind="ExternalOutput").ap()

    with tile.TileContext(nc) as tc, ExitStack() as ctx:
        _kernel_body(ctx, tc, out, xT, maskT, consts, s1d)
    nc.compile()
    return nc


def _kernel_body(ctx, tc, out, xT, maskT, consts, s1d):
    nc = tc.nc
    add, mult, amax = mybir.AluOpType.add, mybir.AluOpType.mult, mybir.AluOpType.max

    const = ctx.enter_context(tc.tile_pool(name="const", bufs=1))
    xpool = ctx.enter_context(tc.tile_pool(name="xpool", bufs=NB))
    mpool = ctx.enter_context(tc.tile_pool(name="mpool", bufs=NB))
    spool = ctx.enter_context(tc.tile_pool(name="spool", bufs=NB))
    s2tpool = ctx.enter_context(tc.tile_pool(name="s2tpool", bufs=2 * NCH))
    upool = ctx.enter_context(tc.tile_pool(name="upool", bufs=5))
    epool = ctx.enter_context(tc.tile_pool(name="epool", bufs=5))
    s1bpool = ctx.enter_context(tc.tile_pool(name="s1bpool", bufs=6))
    hpool = ctx.enter_context(tc.tile_pool(name="hpool", bufs=2 * NCH))
    apool = ctx.enter_context(tc.tile_pool(name="apool", bufs=2))
    evpool = ctx.enter_context(tc.tile_pool(name="evpool", bufs=6))
    npool = ctx.enter_context(tc.tile_pool(name="npool", bufs=4))
    # PSUM: 2 (setup scratch) + 6 (agg out; 2 i-tiles per bank) = 8 banks
    pset = ctx.enter_context(tc.tile_pool(name="pset", bufs=2, space="PSUM"))
    pout = ctx.enter_context(tc.tile_pool(name="pout", bufs=4, space="PSUM"))

    # --- constants (one packed DMA) ----------------------------------------
    cst = const.tile([P, CONST_COLS], F16)
    nc.sync.dma_start(out=cst, in_=consts)
    W_sb = cst[:, C_W : C_W + FOUT]
    WT_sb = cst[:, C_WT : C_WT + FOUT]
    aT_sb = cst[:, C_AT : C_AT + 2 * H]
    ab_sb = cst[0:H, C_AB : C_AB + 1]
    ident8 = cst[0:H, C_ID8 : C_ID8 + H]
    onesrow = cst[0:1, C_ONES : C_ONES + P]
    ident128 = cst[:, C_I128 : C_I128 + P]

    # GPSIMD ucode warmup: first use of a freshly-loaded Q7 kernel pays a
    # ~6us IRAM load; trigger it during the prologue on dummy tiles.
    wrm = const.tile([P, 2], F32)
    nc.vector.memset(wrm[:], 1.0)
    nc.gpsimd.normalize_recip(wrm[:, 0:1], wrm[:, 0:1], wrm[:, 1:2])
    nc.gpsimd.tensor_add(wrm[:, 0:1], wrm[:, 0:1], wrm[:, 1:2])

    # Wa[fin, 0:8]=W@a1^T, [fin, 8:16]=W@a2^T  (shared across graphs)
    p_wa = pset.tile([P, D], F32, tag="setup")
    nc.tensor.matmul(p_wa[:, 0 : 2 * H], WT_sb, aT_sb, start=True, stop=True)
    Wa_sb = const.tile([FIN, 2 * H], F16)
    nc.vector.tensor_copy(Wa_sb[:], p_wa[:, 0 : 2 * H])
    ab32 = const.tile([H, 1], F32)
    nc.vector.tensor_copy(ab32[:], ab_sb)

    G = []  # per-graph setup state
    for b in range(NB):
        # --- per-graph setup ----------------------------------------------
        x_sb = xpool.tile([FIN, D], F16, tag="x")
        nc.sync.dma_start(out=x_sb, in_=xT[b])
        m_sb = mpool.tile([P, NCH * D], F16, tag="mask")
        qs = [nc.scalar, nc.gpsimd, nc.sync]
        for c in range(NCH):
            qs[(b * NCH + c) % 3].dma_start(
                out=m_sb[:, bass.ts(c, D)], in_=maskT[b][:, bass.ts(c, D)]
            )

        # s1/s2 for all heads: [8, D] each
        p_s1 = pset.tile([P, D], F32, tag="setup")
        nc.tensor.matmul(p_s1[0:H, :], Wa_sb[:, 0:H], x_sb[:], start=True, stop=True)
        s1_sb = spool.tile([H, D], F16, tag="s1")
        nc.vector.tensor_copy(s1_sb[:], p_s1[0:H, :])
        # stage s1 rows in DRAM; the head loop row-broadcasts them back via DMA
        nc.scalar.dma_start(out=s1d[b], in_=s1_sb[:])
        mx1 = spool.tile([H, 1], F32, tag="mx1")
        nc.vector.reduce_max(
            out=mx1[:], in_=p_s1[0:H, :], axis=mybir.AxisListType.X, negate=True
        )

        p_s2 = pset.tile([P, D], F32, tag="setup")
        nc.tensor.matmul(
            p_s2[0:H, :], Wa_sb[:, H : 2 * H], x_sb[:], start=True, stop=True
        )
        s2b_sb = spool.tile([H, D], F16, tag="s2")
        nc.vector.tensor_scalar(
            out=s2b_sb[:], in0=p_s2[0:H, :], scalar1=ab32[:], scalar2=None,
            op0=add,
        )
        mx2 = spool.tile([H, 1], F32, tag="mx2")
        nc.vector.reduce_max(
            out=mx2[:], in_=s2b_sb[:], axis=mybir.AxisListType.X, negate=True
        )

        # Per-head negated logit upper bound + 8: exp bias (softmax-shift
        # errors cancel per head, so fp16 is fine here).
        nbound = spool.tile([H, 1], F32, tag="nbound")
        nc.vector.tensor_add(nbound[:], mx1[:], mx2[:])
        nc.vector.tensor_scalar_add(nbound[:], nbound[:], 8.0)
        nb16 = spool.tile([H, 1], F16, tag="nb16")
        nc.vector.tensor_copy(nb16[:], nbound[:])
        # broadcast -bound to [P, H] columns: transpose to a row, then
        # ones-row outer-product
        p_nt = pset.tile([P, D], F32, tag="setup")
        nc.tensor.matmul(p_nt[0:1, 0:H], nb16[:], ident8, start=True, stop=True)
        nbT = spool.tile([1, H], F16, tag="nbT")
        nc.vector.tensor_copy(nbT[:], p_nt[0:1, 0:H])
        p_nb = pset.tile([P, D], F32, tag="setup")
        nc.tensor.matmul(p_nb[:, 0:H], onesrow, nbT[:], start=True, stop=True)
        nbcols = spool.tile([P, H], F16, tag="nbcols")
        nc.vector.tensor_copy(nbcols[:], p_nb[:, 0:H])

        # s2b columns: [P, H] per j-chunk (PE transpose of [8, 128] slices)
        s2bT = []
        for c in range(NCH):
            p_t = pset.tile([P, D], F16, tag="setup")
            nc.tensor.transpose(p_t[:, 0:H], s2b_sb[:, bass.ts(c, P)], ident8)
            st = s2tpool.tile([P, H], F32, tag="s2T")
            nc.vector.tensor_copy(st[:], p_t[:, 0:H])
            s2bT.append(st)

        # h tiles + ones column, fp16, h pre-scaled by 1/H
        haug = []
        for c in range(NCH):
            p_h = pset.tile([P, D], F32, tag="setup")
            nc.tensor.matmul(
                p_h[:, 0:FOUT], x_sb[:, bass.ts(c, P)], W_sb, start=True, stop=True
            )
            ha = hpool.tile([P, FOUT + 1], F16, tag="haug")
            nc.scalar.activation(
                ha[:, 0:FOUT], p_h[:, 0:FOUT],
                mybir.ActivationFunctionType.Copy, scale=1.0 / H,
            )
            nc.vector.memset(ha[:, FOUT : FOUT + 1], 1.0)
            haug.append(ha)

        # head-0 s1b via PE row-broadcast (skips the DRAM round-trip latency
        # that would otherwise gate the first head-graph)
        p_b = pset.tile([P, D], F32, tag="setup")
        nc.tensor.matmul(p_b[:], onesrow, s1_sb[0:1, :], start=True, stop=True)
        s1b0 = s1bpool.tile([P, D], F16, tag="s1b")
        nc.scalar.activation(s1b0[:], p_b[:], mybir.ActivationFunctionType.Copy)
        G.append(dict(m_sb=m_sb, s2bT=s2bT, haug=haug, nbcols=nbcols,
                      s1b0=s1b0))

    # per-graph head-accumulators: one full PSUM bank each, reusing the two
    # setup banks (setup is complete by the first write)
    for b in range(NB):
        acc_ps = pset.tile([P, NCH * FOUT], F32, tag="accps")
        G[b]["acc_ps"] = acc_ps

    # --- main per-head loop, graphs interleaved for deeper ILP ------------
    for hd in range(H):
        for b in range(NB):
            m_sb, s2bT = G[b]["m_sb"], G[b]["s2bT"]
            haug, nbcols = G[b]["haug"], G[b]["nbcols"]
            if hd == 0:
                s1b = G[b]["s1b0"]
            else:
                # S1B = s1 row hd broadcast across partitions (DMA row-bcast)
                s1b = s1bpool.tile([P, D], F16, tag="s1b")
                s1row = s1d[b, hd]
                nc.sync.dma_start(
                    out=s1b[:],
                    in_=bass.AP(
                        tensor=s1d.tensor, offset=s1row.offset,
                        ap=[[0, P], s1row.ap[-1]],
                    ),
                )

            # u = leaky_relu(maskT + s2b[j] + S1B): one fused custom DVE op
            # per j-chunk (replaces STT + separate lrelu pass)
            u = upool.tile([P, NCH * D], F16, tag="u")
            for c in range(NCH):
                _emit_vlrelu(
                    nc,
                    out=u[:, bass.ts(c, D)],
                    in0=m_sb[:, bass.ts(c, D)],
                    in1=s1b[:],
                    s0=s2bT[c][:, hd : hd + 1],
                    imm2=0.01,
                )
            # E = exp(u - bound + 8), one wide fp16 ACT pass
            E = epool.tile([P, NCH * D], F16, tag="E")
            nc.scalar.activation(
                E[:], u[:], mybir.ActivationFunctionType.Exp,
                bias=nbcols[:, hd : hd + 1],
            )

            # agg: psum[i-tile t] += E^T[:, t]^T @ [h/8 | 1]; two i-tiles
            # share a PSUM bank so the f32 evict is 2 wide ACT copies.
            TW = FOUT + 1
            p_bank = []
            for t2 in range(NCH // 2):
                p_o = pout.tile([P, 2 * TW], F32, tag="po")
                for half in range(2):
                    t = 2 * t2 + half
                    for c in range(NCH):
                        nc.tensor.matmul(
                            p_o[:, half * TW : (half + 1) * TW],
                            E[:, c * D + t * P : c * D + (t + 1) * P],
                            haug[c][:],
                            start=(c == 0),
                            stop=(c == NCH - 1),
                        )
                p_bank.append(p_o)
            # batched strided recip per PSUM bank (rowsum cols 128, 257);
            # normalize-evict split DVE/ACT; head-accumulate on GPSIMD
            rcs = []
            for t2 in range(NCH // 2):
                rc = npool.tile([P, 2], F32, tag="rc")
                p_o = p_bank[t2]
                nc.vector.reciprocal(
                    rc[:],
                    bass.AP(
                        tensor=p_o.tensor, offset=p_o[:, FOUT : FOUT + 1].offset,
                        ap=[p_o.ap[0], [TW, 2]],
                    ),
                )
                rcs.append(rc)
            tgt = evpool.tile([P, NCH * FOUT], F16, tag="ev")
            for t in range(NCH):
                p_o = p_bank[t // 2]
                half = t % 2
                if t < 2:
                    nc.vector.tensor_scalar(
                        out=tgt[:, bass.ts(t, FOUT)],
                        in0=p_o[:, half * TW : half * TW + FOUT],
                        scalar1=rcs[t // 2][:, half : half + 1],
                        scalar2=None, op0=mult,
                    )
                else:
                    nc.scalar.activation(
                        tgt[:, bass.ts(t, FOUT)],
                        p_o[:, half * TW : half * TW + FOUT],
                        mybir.ActivationFunctionType.Copy,
                        scale=rcs[t // 2][:, half : half + 1],
                    )
            # head-accumulate in PSUM: acc += I @ ev (one matmul, off the
            # saturated DVE/ACT engines)
            nc.tensor.matmul(
                G[b]["acc_ps"][:], ident128, tgt[:],
                start=(hd == 0), stop=(hd == H - 1),
            )

    for b in range(NB):
        # out[b, t*128+p, f] = acc[p, t*FOUT+f]
        acc_sb = apool.tile([P, NCH * FOUT], F16, tag="acc")
        nc.scalar.activation(
            acc_sb[:], G[b]["acc_ps"][:], mybir.ActivationFunctionType.Copy
        )
        nc.sync.dma_start(
            out=bass.AP(
                tensor=out.tensor, offset=out[b].offset,
                ap=[[FOUT, P], [P * FOUT, NCH], [1, FOUT]],
            ),
            in_=acc_sb[:],
        )


def _prep_core_inputs(input, adj, W, a_w, a_b, core):
    gs = slice(core * NB, (core + 1) * NB)
    x_c = input[gs]                                   # [NB, D, FIN]
    adj_c = adj[gs]                                   # [NB, D, D] int32
    xT = np.ascontiguousarray(x_c.transpose(0, 2, 1)).astype(np.float16)
    adjT = adj_c.transpose(0, 2, 1)                   # [NB, j, i]

    maskT = np.where(adjT > 0, np.float16(0.0), np.float16(NEGM))
    # [NB, j, i] -> [NB, NCH, P, i] -> [NB, P, NCH, i] -> [NB, P, NCH*D]
    maskT = np.ascontiguousarray(
        maskT.reshape(NB, NCH, P, D).transpose(0, 2, 1, 3).reshape(NB, P, NCH * D)
    )
    return {
        "xT": xT,
        "maskT": maskT,
        "consts": _pack_consts(W, a_w, a_b),
    }


def _pack_consts(W, a_w, a_b):
    c = np.zeros((P, CONST_COLS), dtype=np.float16)
    c[:, C_W : C_W + FOUT] = W
    c[:, C_WT : C_WT + FOUT] = W.T
    c[:, C_AT : C_AT + H] = a_w[:, :FOUT].T
    c[:, C_AT + H : C_AT + 2 * H] = a_w[:, FOUT:].T
    c[0:H, C_AB] = a_b
    c[0:H, C_ID8 : C_ID8 + H] = np.eye(H)
    c[0:1, C_ONES : C_ONES + P] = 1.0
    c[:, C_I128 : C_I128 + P] = np.eye(P)
    return c


def get_nc():
    if "nc" not in _NC_CACHE:
        _NC_CACHE["nc"] = _build_bass()
    return _NC_CACHE["nc"]


def run_on_device(in_maps, **kwargs):
    return run_bass_kernel_spmd(get_nc(), in_maps, list(range(NCORES)), **kwargs)


def kernel(input, adj, W, a_w, a_b):
    input = np.asarray(input, dtype=np.float32)
    adj = np.asarray(adj)
    W = np.asarray(W, dtype=np.float32)
    a_w = np.asarray(a_w, dtype=np.float32)
    a_b = np.asarray(a_b, dtype=np.float32)

    in_maps = [
        _prep_core_inputs(input, adj, W, a_w, a_b, c) for c in range(NCORES)
    ]
    res = run_on_device(in_maps)
    outs = [res.results[c]["out"] for c in range(NCORES)]
    return np.concatenate(outs, axis=0).astype(np.float32)


if __name__ == "__main__":
    nc = get_nc()
    print("built ok")


# revision 25
# speedup vs baseline: 1.0401x; 1.0296x over previous
"""GAT layer (nn_GATLayer_44220983279640) — Trainium2 Bass/Tile kernel.

Reference math per graph (B=16, D=512, FIN=FOUT=128, H=8):
    h  = x @ W                                         [D, F]
    s1[hd,i] = h[i] . a1[hd]   s2[hd,j] = h[j] . a2[hd]
    e  = leaky_relu(s1[:,None] + s2[None,:] + ab)      [H, D, D]
    att = softmax_j(where(adj > 0, e, -9e15))
    out = mean_hd(att @ h)                             [D, F]

Sharding: data-parallel over batch, 2 graphs per core on 8 cores.

Device strategy (per graph b, per head hd; E^T[j, i] layout throughout):
  * custom fused DVE op VLRELU: u = leaky_relu(mask + s2[j] + s1b) — one
    DVE pass per j-chunk replacing what would be a scalar_tensor_tensor
    (which has no 2x uop) plus a separate leaky_relu pass.
  * hand-built 2x_1P uop program for VLRELU (lower() only emits 1x): lo
    element on ALU blocks 0-3, hi element on blocks 4-7, injected via
    dve_ops._COMPILE_CACHE and emitted with perf_max=1. Packed fp16
    operands then run at 2 elem/cycle/lane (~480ns per [128,512] chunk).
  * exp on ACT as one wide [128, 2048] fp16 pass, shifted per head so
    outputs are in (0, e^8] (fp16-normal); shift errors cancel in softmax.
  * fp16 mask (0 / -6e4) prepared host-side; s1 row-broadcast via stride-0
    DMA from DRAM staging (head 0 via PE ones-outer-product to skip the
    round-trip at startup).
  * agg: psum += E^T-slice^T @ [h/8 | ones], two i-tiles per PSUM bank;
    rowsum reciprocals as one strided [128,2] DVE op per bank; normalize-
    evict split DVE/ACT (all-DVE for the last heads, where ACT is the
    drain bottleneck); head-accumulate as identity-weight matmuls into two
    dedicated PSUM banks (no DVE/ACT/GPSIMD cost).
  * all setup evictions on DVE (ACT-idle prologue), fp16 setup matmuls,
    mask DMA split across the three DMA queues, fp16 output (host upcasts).

Measured on trn2 (8 cores): HW exec ~79.7us (baseline 102.9us), rel err
1.5e-3 (gate 2e-2). Steady state runs DVE and ACT at ~97-100%; remaining
headroom is the ~17us prologue (7us fixed framework preamble + input DMA
and setup chain) and the ACT-limited pipeline drain.
"""

from contextlib import ExitStack

import numpy as np

import concourse.bass as bass
import concourse.bacc as bacc
import concourse.tile as tile
from concourse import mybir
from concourse import dve_ops as _dvo
from concourse.bass_utils import run_bass_kernel_spmd
from concourse.dve_spec import C0, C2, Spec, Src0, Src1, lower, maxx
from concourse.dve_uop import DveOpSpec


def _register_vlrelu():
    """Custom fused DVE op: out = leaky_relu(in0 + s0 + in1).

    One 1x DVE pass replaces the STT (mask + s2 + s1b) AND the leaky_relu
    pass (scalar_tensor_tensor has no 2x uop, so two stock passes would cost
    ~2x this single fused op)."""
    name = "GAT_VLRELU_ANT"
    for op in _dvo.OPS:
        if op.name == name:
            return op
    x = (Src0 + C0) + Src1

    def _ref(in0, in1, c0, c1, c2):
        y = in0.astype(np.float32) + c0 + in1.astype(np.float32)
        return np.maximum(y, y * c2)

    spec = Spec(body=maxx(x, x * C2), reference=_ref)
    row = _dvo._CUSTOM_DVE_ROW_BASE + len(_dvo.OPS)
    shas = {}
    for ver in ("v3", "v4"):
        try:
            uops = lower(spec, ver=ver)
            shas[ver] = DveOpSpec(
                name=name, opcode=row, uops=uops, rd1_en=True
            ).sha(ver)
        except Exception:
            pass
    op = _dvo.DveOp(name, spec, subdim=False, uops_sha=shas)
    _dvo.OPS.append(op)
    _dvo._SUB_OPCODE_FOR_NAME[name] = row
    return op


VLRELU = _register_vlrelu()


def _vlrelu_uops_2x():
    """Hand-built 2x_1P uop program for VLRELU (lower() only emits 1x).

    Packed fp16 pairs: blocks 0-3 compute the lo element, blocks 4-7 the hi
    element. Inputs ride delay lanes (input lane k+1 <-> delay lane k):
    D0=SRC_0 D1=CONST_0 D2=SRC_1 D3=CONST_2 D4=SRC_0_HI D5=SRC_1_HI.
    u_lo is captured into D0 at block 4 and written from DELAY_0; u_hi is
    block 7's ALU_OUT."""
    from concourse.dve_uop import (
        AluInp, AluOp, DelayInp, InpSel, OutPath, OutSel, Trigger,
        UopConfig, UopDpConfig,
    )

    def blk(op, a, b, cap=None):
        d = UopDpConfig(
            op=op, alu_src0=a, alu_src1=b,
            delay=[DelayInp.PREV_DELAY] * 7,
            alu_out_enable=1,
            delay_enable=[1, 1, 1, 1, 1, 1, 0],
        )
        if cap is not None:
            d.delay[cap] = DelayInp.PREV_ALU_OUT
        return d

    A = AluInp
    dp = [
        blk(AluOp.ADD, A.PREV_DELAY_0, A.PREV_DELAY_1),           # S0+C0
        blk(AluOp.ADD, A.PREV_ALU_OUT, A.PREV_DELAY_2),           # +S1 = x_lo
        blk(AluOp.MULTIPLY, A.PREV_ALU_OUT, A.PREV_DELAY_3, cap=0),  # x_lo*C2; D0<-x_lo
        blk(AluOp.MAX, A.PREV_DELAY_0, A.PREV_ALU_OUT),           # u_lo
        blk(AluOp.ADD, A.PREV_DELAY_4, A.PREV_DELAY_1, cap=0),    # S0H+C0; D0<-u_lo
        blk(AluOp.ADD, A.PREV_ALU_OUT, A.PREV_DELAY_5),           # +S1H = x_hi
        blk(AluOp.MULTIPLY, A.PREV_ALU_OUT, A.PREV_DELAY_3, cap=1),  # x_hi*C2; D1<-x_hi
        blk(AluOp.MAX, A.PREV_DELAY_1, A.PREV_ALU_OUT),           # u_hi
    ]
    u = UopConfig(
        inp=[InpSel.ZERO, InpSel.SRC_0, InpSel.CONST_0, InpSel.SRC_1,
             InpSel.CONST_2, InpSel.SRC_0_HI, InpSel.SRC_1_HI, InpSel.ZERO],
        inp_enable=[0, 1, 1, 1, 1, 1, 1, 0],
        out={OutPath.WR0_LO: OutSel.DELAY_0, OutPath.WR0_HI: OutSel.ALU_OUT,
             OutPath.WR1_LO: OutSel.ALU_OUT, OutPath.WR1_HI: OutSel.ALU_OUT},
        out_enable={OutPath.WR0_LO: 1, OutPath.WR0_HI: 1,
                    OutPath.WR1_LO: 0, OutPath.WR1_HI: 0},
        require_inp0=1, require_inp1=1,
        trigger=(Trigger.SRC_TENSOR_DONE, Trigger.NONE, Trigger.NONE),
        datapath_config=dp,
    )
    return [u]


def _inject_vlrelu_2x():
    """Swap the compiled spec for VLRELU with one carrying the 2x program.

    DveOp.compile() is memoised in dve_ops._COMPILE_CACHE; seeding the cache
    makes dve_table_for_ops pick up the augmented table."""
    from concourse.dve_table_gen import dve_ver_for

    ver = dve_ver_for("TRN2")
    spec1x = lower(VLRELU.spec, ver=ver)
    augmented = DveOpSpec(
        name=VLRELU.name,
        opcode=_dvo.get_dve_sub_opcode(VLRELU.name),
        uops=spec1x,
        uops_2x=_vlrelu_uops_2x(),
        rd1_en=True,
        perf_max=1,
    )
    augmented.validate(ver)
    _dvo._COMPILE_CACHE[(VLRELU.name, ver)] = augmented


_inject_vlrelu_2x()


def _emit_vlrelu(nc, out, in0, in1, s0, imm2):
    """nc.vector._custom_dve for VLRELU, but with perf_max=1 so the engine
    may select the 2x_1P table slot when operands are packed fp16."""
    import concourse.bass_isa as bass_isa

    v = nc.vector
    if VLRELU.name not in v.bass.m.ant_custom_dve_ops:
        v.bass.m.ant_custom_dve_ops = sorted(
            {*v.bass.m.ant_custom_dve_ops, VLRELU.name}
        )
    shape = bass_isa.CustomDveShape.TTSS
    isa_opcode = v.bass.isa.Opcode[
        f"NEURON_ISA_TPB_OPCODE_CUSTOM_DVE_ANT_{shape.slot()}"
    ].value
    ins = [
        v.lower_ap(in0, for_isa=True),
        v.lower_ap(in1, for_isa=True),
        v.lower_ap(s0, for_isa=True),
        mybir.ImmediateValue(dtype=mybir.dt.float32, value=0.0),
    ]
    outs = [v.lower_ap(out, for_isa=True)]
    return v.add_instruction(
        bass_isa.InstCustomDveAnt(
            name=v.bass.get_next_instruction_name(),
            op_name=VLRELU.name,
            rd1_en=True,
            subdim=0,
            imm2=imm2,
            shape=shape,
            row=_dvo.get_dve_sub_opcode(VLRELU.name),
            isa_opcode=isa_opcode,
            perf_max=1,
            ins=ins,
            outs=outs,
        )
    )

B, D, FIN, FOUT, H = 16, 512, 128, 128, 8
NCORES = 8
NB = B // NCORES          # graphs per core
P = 128                   # partitions
NCH = D // P              # 4 j-chunks / i-tiles
NEGM = -60000.0           # fp16-safe "masked" logit

F32 = mybir.dt.float32
F16 = mybir.dt.float16

# packed fp16 consts layout (columns): W | W^T | aT | ab | id8 | ones | zero
C_W = 0
C_WT = FOUT
C_AT = 2 * FOUT
C_AB = 2 * FOUT + 2 * H
C_ID8 = C_AB + 1
C_ONES = C_ID8 + H
C_I128 = C_ONES + P
C_ZERO = C_I128 + P
CONST_COLS = C_ZERO + 1

_NC_CACHE = {}


def _build_bass():
    nc = bacc.Bacc("TRN2", debug=False, num_devices=NCORES)

    xT = nc.dram_tensor("xT", [NB, FIN, D], F16, kind="ExternalInput").ap()
    maskT = nc.dram_tensor("maskT", [NB, P, NCH * D], F16, kind="ExternalInput").ap()
    consts = nc.dram_tensor("consts", [P, CONST_COLS], F16, kind="ExternalInput").ap()
    s1d = nc.dram_tensor("s1d", [NB, H, D], F16).ap()
    out = nc.dram_tensor("out", [NB, D, FOUT], F16, kind="ExternalOutput").ap()

    with tile.TileContext(nc) as tc, ExitStack() as ctx:
        _kernel_body(ctx, tc, out, xT, maskT, consts, s1d)
    nc.compile()
    return nc


def _kernel_body(ctx, tc, out, xT, maskT, consts, s1d):
    nc = tc.nc
    add, mult, amax = mybir.AluOpType.add, mybir.AluOpType.mult, mybir.AluOpType.max

    const = ctx.enter_context(tc.tile_pool(name="const", bufs=1))
    xpool = ctx.enter_context(tc.tile_pool(name="xpool", bufs=NB))
    mpool = ctx.enter_context(tc.tile_pool(name="mpool", bufs=NB))
    spool = ctx.enter_context(tc.tile_pool(name="spool", bufs=NB))
    s2tpool = ctx.enter_context(tc.tile_pool(name="s2tpool", bufs=2 * NCH))
    upool = ctx.enter_context(tc.tile_pool(name="upool", bufs=5))
    epool = ctx.enter_context(tc.tile_pool(name="epool", bufs=5))
    s1bpool = ctx.enter_context(tc.tile_pool(name="s1bpool", bufs=6))
    hpool = ctx.enter_context(tc.tile_pool(name="hpool", bufs=2 * NCH))
    apool = ctx.enter_context(tc.tile_pool(name="apool", bufs=2))
    evpool = ctx.enter_context(tc.tile_pool(name="evpool", bufs=6))
    npool = ctx.enter_context(tc.tile_pool(name="npool", bufs=4))
    # PSUM: 2 (setup scratch) + 6 (agg out; 2 i-tiles per bank) = 8 banks
    pset = ctx.enter_context(tc.tile_pool(name="pset", bufs=2, space="PSUM"))
    pout = ctx.enter_context(tc.tile_pool(name="pout", bufs=4, space="PSUM"))

    # --- constants (one packed DMA) ----------------------------------------
    cst = const.tile([P, CONST_COLS], F16)
    nc.sync.dma_start(out=cst, in_=consts)
    W_sb = cst[:, C_W : C_W + FOUT]
    WT_sb = cst[:, C_WT : C_WT + FOUT]
    aT_sb = cst[:, C_AT : C_AT + 2 * H]
    ab_sb = cst[0:H, C_AB : C_AB + 1]
    ident8 = cst[0:H, C_ID8 : C_ID8 + H]
    onesrow = cst[0:1, C_ONES : C_ONES + P]
    ident128 = cst[:, C_I128 : C_I128 + P]

    # GPSIMD ucode warmup: first use of a freshly-loaded Q7 kernel pays a
    # ~6us IRAM load; trigger it during the prologue on dummy tiles.
    wrm = const.tile([P, 2], F32)
    nc.vector.memset(wrm[:], 1.0)
    nc.gpsimd.normalize_recip(wrm[:, 0:1], wrm[:, 0:1], wrm[:, 1:2])
    nc.gpsimd.tensor_add(wrm[:, 0:1], wrm[:, 0:1], wrm[:, 1:2])

    # Wa[fin, 0:8]=W@a1^T, [fin, 8:16]=W@a2^T  (shared across graphs)
    p_wa = pset.tile([P, D], F32, tag="setup")
    nc.tensor.matmul(p_wa[:, 0 : 2 * H], WT_sb, aT_sb, start=True, stop=True)
    Wa_sb = const.tile([FIN, 2 * H], F16)
    nc.vector.tensor_copy(Wa_sb[:], p_wa[:, 0 : 2 * H])
    ab32 = const.tile([H, 1], F32)
    nc.vector.tensor_copy(ab32[:], ab_sb)

    G = []  # per-graph setup state
    for b in range(NB):
        # --- per-graph setup ----------------------------------------------
        x_sb = xpool.tile([FIN, D], F16, tag="x")
        nc.sync.dma_start(out=x_sb, in_=xT[b])
        m_sb = mpool.tile([P, NCH * D], F16, tag="mask")
        qs = [nc.scalar, nc.gpsimd, nc.sync]
        for c in range(NCH):
            qs[(b * NCH + c) % 3].dma_start(
                out=m_sb[:, bass.ts(c, D)], in_=maskT[b][:, bass.ts(c, D)]
            )

        # s1/s2 for all heads: [8, D] each
        p_s1 = pset.tile([P, D], F32, tag="setup")
        nc.tensor.matmul(p_s1[0:H, :], Wa_sb[:, 0:H], x_sb[:], start=True, stop=True)
        s1_sb = spool.tile([H, D], F16, tag="s1")
        nc.vector.tensor_copy(s1_sb[:], p_s1[0:H, :])
        # stage s1 rows in DRAM; the head loop row-broadcasts them back via DMA
        nc.scalar.dma_start(out=s1d[b], in_=s1_sb[:])
        mx1 = spool.tile([H, 1], F32, tag="mx1")
        nc.vector.reduce_max(
            out=mx1[:], in_=p_s1[0:H, :], axis=mybir.AxisListType.X, negate=True
        )

        p_s2 = pset.tile([P, D], F32, tag="setup")
        nc.tensor.matmul(
            p_s2[0:H, :], Wa_sb[:, H : 2 * H], x_sb[:], start=True, stop=True
        )
        s2b_sb = spool.tile([H, D], F16, tag="s2")
        nc.vector.tensor_scalar(
            out=s2b_sb[:], in0=p_s2[0:H, :], scalar1=ab32[:], scalar2=None,
            op0=add,
        )
        mx2 = spool.tile([H, 1], F32, tag="mx2")
        nc.vector.reduce_max(
            out=mx2[:], in_=s2b_sb[:], axis=mybir.AxisListType.X, negate=True
        )

        # Per-head negated logit upper bound + 8: exp bias (softmax-shift
        # errors cancel per head, so fp16 is fine here).
        nbound = spool.tile([H, 1], F32, tag="nbound")
        nc.vector.tensor_add(nbound[:], mx1[:], mx2[:])
        nc.vector.tensor_scalar_add(nbound[:], nbound[:], 8.0)
        nb16 = spool.tile([H, 1], F16, tag="nb16")
        nc.vector.tensor_copy(nb16[:], nbound[:])
        # broadcast -bound to [P, H] columns: transpose to a row, then
        # ones-row outer-product
        p_nt = pset.tile([P, D], F32, tag="setup")
        nc.tensor.matmul(p_nt[0:1, 0:H], nb16[:], ident8, start=True, stop=True)
        nbT = spool.tile([1, H], F16, tag="nbT")
        nc.vector.tensor_copy(nbT[:], p_nt[0:1, 0:H])
        p_nb = pset.tile([P, D], F32, tag="setup")
        nc.tensor.matmul(p_nb[:, 0:H], onesrow, nbT[:], start=True, stop=True)
        nbcols = spool.tile([P, H], F16, tag="nbcols")
        nc.vector.tensor_copy(nbcols[:], p_nb[:, 0:H])

        # s2b columns: [P, H] per j-chunk (PE transpose of [8, 128] slices)
        s2bT = []
        for c in range(NCH):
            p_t = pset.tile([P, D], F16, tag="setup")
            nc.tensor.transpose(p_t[:, 0:H], s2b_sb[:, bass.ts(c, P)], ident8)
            st = s2tpool.tile([P, H], F32, tag="s2T")
            nc.vector.tensor_copy(st[:], p_t[:, 0:H])
            s2bT.append(st)

        # h tiles + ones column, fp16, h pre-scaled by 1/H
        haug = []
        for c in range(NCH):
            p_h = pset.tile([P, D], F32, tag="setup")
            nc.tensor.matmul(
                p_h[:, 0:FOUT], x_sb[:, bass.ts(c, P)], W_sb, start=True, stop=True
            )
            ha = hpool.tile([P, FOUT + 1], F16, tag="haug")
            nc.scalar.activation(
                ha[:, 0:FOUT], p_h[:, 0:FOUT],
                mybir.ActivationFunctionType.Copy, scale=1.0 / H,
            )
            nc.vector.memset(ha[:, FOUT : FOUT + 1], 1.0)
            haug.append(ha)

        # head-0 s1b via PE row-broadcast (skips the DRAM round-trip latency
        # that would otherwise gate the first head-graph)
        p_b = pset.tile([P, D], F32, tag="setup")
        nc.tensor.matmul(p_b[:], onesrow, s1_sb[0:1, :], start=True, stop=True)
        s1b0 = s1bpool.tile([P, D], F16, tag="s1b")
        nc.scalar.activation(s1b0[:], p_b[:], mybir.ActivationFunctionType.Copy)
        G.append(dict(m_sb=m_sb, s2bT=s2bT, haug=haug, nbcols=nbcols,
                      s1b0=s1b0))

    # per-graph head-accumulators: one full PSUM bank each, reusing the two
    # setup banks (setup is complete by the first write)
    for b in range(NB):
        acc_ps = pset.tile([P, NCH * FOUT], F32, tag="accps")
        G[b]["acc_ps"] = acc_ps

    # --- main per-head loop, graphs interleaved for deeper ILP ------------
    for hd in range(H):
        for b in range(NB):
            m_sb, s2bT = G[b]["m_sb"], G[b]["s2bT"]
            haug, nbcols = G[b]["haug"], G[b]["nbcols"]
            if hd == 0:
                s1b = G[b]["s1b0"]
            else:
                # S1B = s1 row hd broadcast across partitions (DMA row-bcast)
                s1b = s1bpool.tile([P, D], F16, tag="s1b")
                s1row = s1d[b, hd]
                nc.sync.dma_start(
                    out=s1b[:],
                    in_=bass.AP(
                        tensor=s1d.tensor, offset=s1row.offset,
                        ap=[[0, P], s1row.ap[-1]],
                    ),
                )

            # u = leaky_relu(maskT + s2b[j] + S1B): one fused custom DVE op
            # per j-chunk (replaces STT + separate lrelu pass)
            u = upool.tile([P, NCH * D], F16, tag="u")
            for c in range(NCH):
                _emit_vlrelu(
                    nc,
                    out=u[:, bass.ts(c, D)],
                    in0=m_sb[:, bass.ts(c, D)],
                    in1=s1b[:],
                    s0=s2bT[c][:, hd : hd + 1],
                    imm2=0.01,
                )
            # E = exp(u - bound + 8), one wide fp16 ACT pass
            E = epool.tile([P, NCH * D], F16, tag="E")
            nc.scalar.activation(
                E[:], u[:], mybir.ActivationFunctionType.Exp,
                bias=nbcols[:, hd : hd + 1],
            )

            # agg: psum[i-tile t] += E^T[:, t]^T @ [h/8 | 1]; two i-tiles
            # share a PSUM bank so the f32 evict is 2 wide ACT copies.
            TW = FOUT + 1
            p_bank = []
            for t2 in range(NCH // 2):
                p_o = pout.tile([P, 2 * TW], F32, tag="po")
                for half in range(2):
                    t = 2 * t2 + half
                    for c in range(NCH):
                        nc.tensor.matmul(
                            p_o[:, half * TW : (half + 1) * TW],
                            E[:, c * D + t * P : c * D + (t + 1) * P],
                            haug[c][:],
                            start=(c == 0),
                            stop=(c == NCH - 1),
                        )
                p_bank.append(p_o)
            # batched strided recip per PSUM bank (rowsum cols 128, 257);
            # normalize-evict split DVE/ACT; head-accumulate on GPSIMD
            rcs = []
            for t2 in range(NCH // 2):
                rc = npool.tile([P, 2], F32, tag="rc")
                p_o = p_bank[t2]
                nc.vector.reciprocal(
                    rc[:],
                    bass.AP(
                        tensor=p_o.tensor, offset=p_o[:, FOUT : FOUT + 1].offset,
                        ap=[p_o.ap[0], [TW, 2]],
                    ),
                )
                rcs.append(rc)
            tgt = evpool.tile([P, NCH * FOUT], F16, tag="ev")
            for t in range(NCH):
                p_o = p_bank[t // 2]
                half = t % 2
                if t < 2 or hd >= 5:
                    nc.vector.tensor_scalar(
                        out=tgt[:, bass.ts(t, FOUT)],
                        in0=p_o[:, half * TW : half * TW + FOUT],
                        scalar1=rcs[t // 2][:, half : half + 1],
                        scalar2=None, op0=mult,
                    )
                else:
                    nc.scalar.activation(
                        tgt[:, bass.ts(t, FOUT)],
                        p_o[:, half * TW : half * TW + FOUT],
                        mybir.ActivationFunctionType.Copy,
                        scale=rcs[t // 2][:, half : half + 1],
                    )
            # head-accumulate in PSUM: acc += I @ ev (one matmul, off the
            # saturated DVE/ACT engines)
            nc.tensor.matmul(
                G[b]["acc_ps"][:], ident128, tgt[:],
                start=(hd == 0), stop=(hd == H - 1),
            )

    for b in range(NB):
        # out[b, t*128+p, f] = acc[p, t*FOUT+f]
        acc_sb = apool.tile([P, NCH * FOUT], F16, tag="acc")
        nc.scalar.activation(
            acc_sb[:], G[b]["acc_ps"][:], mybir.ActivationFunctionType.Copy
        )
        nc.sync.dma_start(
            out=bass.AP(
                tensor=out.tensor, offset=out[b].offset,
                ap=[[FOUT, P], [P * FOUT, NCH], [1, FOUT]],
            ),
            in_=acc_sb[:],
        )


def _prep_core_inputs(input, adj, W, a_w, a_b, core):
    gs = slice(core * NB, (core + 1) * NB)
    x_c = input[gs]                                   # [NB, D, FIN]
    adj_c = adj[gs]                                   # [NB, D, D] int32
    xT = np.ascontiguousarray(x_c.transpose(0, 2, 1)).astype(np.float16)
    adjT = adj_c.transpose(0, 2, 1)                   # [NB, j, i]

    maskT = np.where(adjT > 0, np.float16(0.0), np.float16(NEGM))
    # [NB, j, i] -> [NB, NCH, P, i] -> [NB, P, NCH, i] -> [NB, P, NCH*D]
    maskT = np.ascontiguousarray(
        maskT.reshape(NB, NCH, P, D).transpose(0, 2, 1, 3).reshape(NB, P, NCH * D)
    )
    return {
        "xT": xT,
        "maskT": maskT,
        "consts": _pack_consts(W, a_w, a_b),
    }


def _pack_consts(W, a_w, a_b):
    c = np.zeros((P, CONST_COLS), dtype=np.float16)
    c[:, C_W : C_W + FOUT] = W
    c[:, C_WT : C_WT + FOUT] = W.T
    c[:, C_AT : C_AT + H] = a_w[:, :FOUT].T
    c[:, C_AT + H : C_AT + 2 * H] = a_w[:, FOUT:].T
    c[0:H, C_AB] = a_b
    c[0:H, C_ID8 : C_ID8 + H] = np.eye(H)
    c[0:1, C_ONES : C_ONES + P] = 1.0
    c[:, C_I128 : C_I128 + P] = np.eye(P)
    return c


def get_nc():
    if "nc" not in _NC_CACHE:
        _NC_CACHE["nc"] = _build_bass()
    return _NC_CACHE["nc"]


def run_on_device(in_maps, **kwargs):
    return run_bass_kernel_spmd(get_nc(), in_maps, list(range(NCORES)), **kwargs)


def kernel(input, adj, W, a_w, a_b):
    input = np.asarray(input, dtype=np.float32)
    adj = np.asarray(adj)
    W = np.asarray(W, dtype=np.float32)
    a_w = np.asarray(a_w, dtype=np.float32)
    a_b = np.asarray(a_b, dtype=np.float32)

    in_maps = [
        _prep_core_inputs(input, adj, W, a_w, a_b, c) for c in range(NCORES)
    ]
    res = run_on_device(in_maps)
    outs = [res.results[c]["out"] for c in range(NCORES)]
    return np.concatenate(outs, axis=0).astype(np.float32)


if __name__ == "__main__":
    nc = get_nc()
    print("built ok")
